# revision 12
# baseline (speedup 1.0000x reference)
"""Expert-parallel MoE FFN kernel for 8 trn2 NeuronCores.

Problem (per full input):
  x [4, 8, 512, 1024], audio_feat [4, 256, 1024],
  W1/Wa [8, 1024, 4096], b1 [8, 4096], W2 [8, 4096, 1024], b2 [8, 1024]
  out[b,e,n,:] = gelu_tanh(x[b,e,n] @ W1[e] + b1[e] + mean(audio_feat[b]) @ Wa[e]) @ W2[e] + b2[e]

Sharding: expert-parallel — core e owns expert e (weights + x[:, e] slice);
pooled audio replicated. No collectives needed: shard/gather on host.

Per-core kernel (all matmul operands bf16 — same PE rate as f32r but half
the DMA/SBUF and 2x faster weight loads via FWL; PSUM/accumulator fp32):
  - single pass over all 2048 tokens (weights stream exactly once)
  - dff is blocked 8x512; per block:
      audio_h block   = Wa_blk^T @ pooled^T  (stationary wa chunks, moving
                        [128,4] pooled — lands directly in [dff, b] layout)
      GEMM1           h^T tiles [128 dff, 512 tok]; token-major inner loop
                      so the gelu (ACT) of token-chunk tb drains its PSUM
                      bank 24 matmuls before the next c-chunk needs it
      GEMM2           one [128 tok, 1024 d] PSUM tile (2 banks) per token
                      tile; a single DVE add accumulates it into the SBUF
                      fp32 accumulator (halves DVE instruction pressure)
  - x is staged as 4 per-token-block tiles so GEMM1 starts on block 0
    as soon as the first 1MB of x has landed.
"""
from contextlib import ExitStack

import ml_dtypes
import numpy as np

import concourse.bass as bass
import concourse.tile as tile
from concourse import bacc, mybir
from concourse.bass_utils import run_bass_kernel_spmd

F32 = mybir.dt.float32
BF16 = mybir.dt.bfloat16
AF = mybir.ActivationFunctionType

B, E, N, D = 4, 8, 512, 1024
DFF = 4 * D
NA = 256
TOK = B * N            # 2048 tokens per expert
KC = D // 128          # 8 d-chunks
NDFB = 8               # dff blocks
DFB = DFF // NDFB      # 512
NCC = DFB // 128       # 4 c-chunks per block
NTB = 4                # token chunks of 512 (== batch b)
NTS = TOK // 128       # 16 token chunks of 128
NC_CORES = 8

_cache = {}


def _build():
    nc = bacc.Bacc("TRN2", target_bir_lowering=False, debug=False,
                   num_devices=NC_CORES)

    xT_d = nc.declare_dram_parameter("xT", [NTB, 128, KC, N], BF16, isOutput=False)
    apT_d = nc.declare_dram_parameter("apT", [128, KC, B], BF16, isOutput=False)
    w1_d = nc.declare_dram_parameter("w1", [NDFB, 128, KC, DFB], BF16, isOutput=False)
    wa_d = nc.declare_dram_parameter("wa", [NDFB, 128, KC, DFB], BF16, isOutput=False)
    w2_d = nc.declare_dram_parameter("w2", [NDFB, 128, NCC, D], BF16, isOutput=False)
    b1t_d = nc.declare_dram_parameter("b1t", [128, DFF // 128], F32, isOutput=False)
    b2b_d = nc.declare_dram_parameter("b2b", [128, D], F32, isOutput=False)
    out_d = nc.declare_dram_parameter("out", [TOK, D], F32, isOutput=True)

    with tile.TileContext(nc) as tc, ExitStack() as ctx:
        sb = ctx.enter_context(tc.tile_pool(name="sb", bufs=1))
        ps = ctx.enter_context(
            tc.tile_pool(name="ps", bufs=1, space=bass.MemorySpace.PSUM))

        # ---- small persistent tiles -------------------------------------
        apT_t = sb.tile([128, KC, B], BF16, name="apT_t")
        b1t_t = sb.tile([128, DFF // 128], F32, name="b1t_t")
        b2b_t = sb.tile([128, D], F32, name="b2b_t")
        baud_t = sb.tile([128, DFF // 128, B], F32, name="baud_t")
        nc.sync.dma_start(out=apT_t[:], in_=apT_d.ap())
        nc.gpsimd.dma_start(out=b1t_t[:], in_=b1t_d.ap())
        nc.gpsimd.dma_start(out=b2b_t[:], in_=b2b_d.ap())

        # ---- DMA helpers (one contiguous 8KB/partition load per call) ---
        def dma_w(which, d_param, blk, shape):
            t = sb.tile(shape, BF16, name=f"{which}_{blk}", tag=which, bufs=2)
            nc.sync.dma_start(out=t[:], in_=d_param.ap()[blk])
            return t

        # ---- start-up: hand-ordered DMA queue ---------------------------
        # The start is DMA-throughput-bound, so block 0's tiles are split
        # fine-grained and ordered exactly along the consumption order of
        # the PE stream: wa0 (audio) -> w1 c0-chunk + xT tb0 (GEMM1 c0/tb0)
        # -> remaining token blocks -> rest of w1 -> w2.
        wa_t = dma_w("wa", wa_d, 0, [128, KC, DFB])
        w1c0_t = sb.tile([128, KC, 128], BF16, name="w1c0_t")
        nc.sync.dma_start(out=w1c0_t[:], in_=w1_d.ap()[0][:, :, 0:128])
        xT_t = []
        xT0h = []
        for h in range(2):
            t = sb.tile([128, KC // 2, N], BF16, name=f"xT_0_{h}")
            nc.sync.dma_start(out=t[:], in_=xT_d.ap()[0][:, h * 4:(h + 1) * 4, :])
            xT0h.append(t)
        xT_t.append(None)  # tb0 handled via xT0h
        for tb in range(1, NTB):
            t = sb.tile([128, KC, N], BF16, name=f"xT_{tb}")
            nc.sync.dma_start(out=t[:], in_=xT_d.ap()[tb])
            xT_t.append(t)
        w1r_t = sb.tile([128, KC, DFB - 128], BF16, name="w1r_t")
        nc.sync.dma_start(out=w1r_t[:], in_=w1_d.ap()[0][:, :, 128:DFB])
        w2_t = dma_w("w2", w2_d, 0, [128, NCC, D])

        def xT_ap(tb, kc):
            if tb == 0:
                return xT0h[kc // 4][:, kc % 4, :]
            return xT_t[tb][:, kc, :]

        def w1_ap(blk, w1t, kc, c):
            if blk == 0:
                if c == 0:
                    return w1c0_t[:, kc, :]
                return w1r_t[:, kc, (c - 1) * 128:c * 128]
            return w1t[:, kc, c * 128:(c + 1) * 128]

        # ---- PE warm-up -------------------------------------------------
        # Cover the DMA-bound first ~7us with throwaway matmuls so the HAM
        # clock gate is released by the time real work arrives; the dummy
        # activation pulls the one-time ~2.6us gelu table load off the
        # first real GEMM1 chunk's critical path.
        scr_t = sb.tile([128, 4], F32, name="scr_t")
        nc.scalar.activation(scr_t[:], apT_t[:, 0, :], AF.Gelu_apprx_tanh,
                             scale=1.0)
        psW = ps.tile([B, B], F32, name="psW", tag="ps2", bufs=2)
        for _ in range(168):
            nc.tensor.matmul(psW[:], apT_t[:, 0, :], apT_t[:, 0, :],
                             start=True, stop=True)

        # ---- main loop --------------------------------------------------
        oacc = [sb.tile([128, D], F32, name=f"oacc_{t}", tag=f"oacc{t}",
                        bufs=1) for t in range(NTS)]
        w1_t = None  # block 0 reads via w1c0_t / w1r_t
        for blk in range(NDFB):
            first_blk = blk == 0
            last_blk = blk == NDFB - 1
            if not first_blk:
                wa_t = dma_w("wa", wa_d, blk, [128, KC, DFB])
                w1_t = dma_w("w1", w1_d, blk, [128, KC, DFB])
                w2_t = dma_w("w2", w2_d, blk, [128, NCC, D])

            # audio bias: baud[:, cg, b] = (Wa_blk^T @ pooled^T)[dff, b] + b1
            for c in range(NCC):
                cg = blk * NCC + c
                psB = ps.tile([128, B], F32, name=f"psB{cg}", tag="ps2",
                              bufs=2)
                for kc in range(KC):
                    nc.tensor.matmul(
                        psB[:], wa_t[:, kc, c * 128:(c + 1) * 128],
                        apT_t[:, kc, :],
                        start=(kc == 0), stop=(kc == KC - 1))
                nc.vector.tensor_scalar_add(
                    baud_t[:, cg, :], psB[:], b1t_t[:, cg:cg + 1])

            # GEMM1: h^T tiles [128 dff, 512 tok]; token-major inner order.
            # Block 0 runs token-OUTER (all c-chunks of tb0 before tb1...)
            # so the PE has 6.9us of work per 1MB x-chunk while the next
            # chunk is still streaming in — no stalls on the DMA ramp.
            hT = [[None] * NTB for _ in range(NCC)]
            if first_blk:
                order = [(tb, c) for tb in range(NTB) for c in range(NCC)]
            else:
                order = [(tb, c) for c in range(NCC) for tb in range(NTB)]
            for tb, c in order:
                cg = blk * NCC + c
                slot = c if first_blk else tb
                p1 = ps.tile([128, N], F32, name=f"ps1_{blk}_{c}_{tb}",
                             tag=f"ps1{slot}", bufs=1)
                for kc in range(KC):
                    nc.tensor.matmul(
                        p1[:], w1_ap(blk, w1_t, kc, c),
                        xT_ap(tb, kc),
                        start=(kc == 0), stop=(kc == KC - 1))
                h = sb.tile([128, N], BF16, name=f"hT_{blk}_{c}_{tb}",
                            tag=f"hT{c}b{tb}", bufs=2)
                nc.scalar.activation(
                    h[:], p1[:], AF.Gelu_apprx_tanh,
                    bias=baud_t[:, cg, tb:tb + 1], scale=1.0)
                hT[c][tb] = h

            # GEMM2: one [128 tok, 1024 d] PSUM tile (2 banks) per tsg
            for tsg in range(NTS):
                tb, r = tsg // 4, tsg % 4
                tail = last_blk and tsg == NTS - 1
                if not tail:
                    p2 = ps.tile([128, D], F32, name=f"ps2_{blk}_{tsg}",
                                 tag="ps2", bufs=2)
                    halves = [p2[:, 0:512], p2[:, 512:1024]]
                    nq, qw = 2, 512
                else:
                    # final tile: four 256-col tiles in GEMM1's (now idle)
                    # banks so each quarter drains+stores while the next
                    # is still on the PE — shortens the post-last-matmul
                    # add+store+receipt chain
                    pt = [ps.tile([128, 256], F32, name=f"ps2t_{q}",
                                  tag=f"ps1{q}", bufs=1) for q in range(4)]
                    halves = [t[:] for t in pt]
                    nq, qw = 4, 256
                for dh in range(nq):
                    for c in range(NCC):
                        nc.tensor.matmul(
                            halves[dh], hT[c][tb][:, r * 128:(r + 1) * 128],
                            w2_t[:, c, dh * qw:(dh + 1) * qw],
                            start=(c == 0), stop=(c == NCC - 1))
                    if tail:
                        dst = oacc[tsg][:, dh * qw:(dh + 1) * qw]
                        nc.vector.tensor_add(dst, dst, halves[dh])
                        row0 = tsg * 128
                        nc.scalar.dma_start(
                            out=out_d.ap()[row0:row0 + 128,
                                           dh * qw:(dh + 1) * qw],
                            in_=dst)
                if not tail:
                    if first_blk:
                        nc.vector.tensor_add(oacc[tsg][:], p2[:], b2b_t[:])
                    else:
                        nc.vector.tensor_add(oacc[tsg][:], oacc[tsg][:], p2[:])
                    if last_blk:
                        row0 = tsg * 128
                        nc.scalar.dma_start(
                            out=out_d.ap()[row0:row0 + 128, :],
                            in_=oacc[tsg][:])

    nc.compile()
    return nc


def _get_nc():
    if "nc" not in _cache:
        _cache["nc"] = _build()
    return _cache["nc"]


def _prep_in_maps(x, audio_feat, W1, b1, Wa, W2, b2):
    bf = ml_dtypes.bfloat16
    pooled = audio_feat.mean(axis=1)                          # [B, D]
    apT = np.ascontiguousarray(
        pooled.T.reshape(KC, 128, B).transpose(1, 0, 2)).astype(bf)
    in_maps = []
    for e in range(E):
        xT = np.ascontiguousarray(
            x[:, e].reshape(TOK, D).astype(bf)
            .reshape(NTB, N, KC, 128).transpose(0, 3, 2, 1))
        w1 = np.ascontiguousarray(
            W1[e].astype(bf).reshape(KC, 128, NDFB, DFB).transpose(2, 1, 0, 3))
        wa = np.ascontiguousarray(
            Wa[e].astype(bf).reshape(KC, 128, NDFB, DFB).transpose(2, 1, 0, 3))
        w2 = np.ascontiguousarray(
            W2[e].astype(bf).reshape(NDFB, NCC, 128, D).transpose(0, 2, 1, 3))
        in_maps.append({
            "xT": xT,
            "apT": apT,
            "w1": w1,
            "wa": wa,
            "w2": w2,
            "b1t": np.ascontiguousarray(b1[e].reshape(DFF // 128, 128).T),
            "b2b": np.ascontiguousarray(np.broadcast_to(b2[e], (128, D))),
        })
    return in_maps


def kernel(x, audio_feat, W1, b1, Wa, W2, b2):
    x = np.asarray(x, dtype=np.float32)
    audio_feat = np.asarray(audio_feat, dtype=np.float32)
    W1 = np.asarray(W1, dtype=np.float32)
    b1 = np.asarray(b1, dtype=np.float32)
    Wa = np.asarray(Wa, dtype=np.float32)
    W2 = np.asarray(W2, dtype=np.float32)
    b2 = np.asarray(b2, dtype=np.float32)

    nc = _get_nc()
    in_maps = _prep_in_maps(x, audio_feat, W1, b1, Wa, W2, b2)
    _cache["in_maps"] = in_maps

    # A prior tenant can leave the accelerator in an unrecoverable state
    # that clears after one failed attempt; retry to absorb that.
    last_err = None
    for attempt in range(3):
        try:
            res = run_bass_kernel_spmd(nc, in_maps, list(range(NC_CORES)))
            break
        except Exception as err:  # noqa: BLE001
            last_err = err
            import time
            time.sleep(2.0)
    else:
        raise last_err

    out = np.empty((B, E, N, D), dtype=np.float32)
    for e in range(E):
        out[:, e] = res.results[e]["out"].reshape(B, N, D)
    return out


# revision 13
# speedup vs baseline: 1.0042x; 1.0042x over previous
"""Expert-parallel MoE FFN kernel for 8 trn2 NeuronCores.

Problem (per full input):
  x [4, 8, 512, 1024], audio_feat [4, 256, 1024],
  W1/Wa [8, 1024, 4096], b1 [8, 4096], W2 [8, 4096, 1024], b2 [8, 1024]
  out[b,e,n,:] = gelu_tanh(x[b,e,n] @ W1[e] + b1[e] + mean(audio_feat[b]) @ Wa[e]) @ W2[e] + b2[e]

Sharding: expert-parallel — core e owns expert e (weights + x[:, e] slice);
pooled audio replicated. No collectives needed: shard/gather on host.

Per-core kernel (all matmul operands bf16 — same PE rate as f32r but half
the DMA/SBUF and 2x faster weight loads via FWL; PSUM/accumulator fp32):
  - single pass over all 2048 tokens (weights stream exactly once)
  - dff is blocked 8x512; per block:
      audio_h block   = Wa_blk^T @ pooled^T  (stationary wa chunks, moving
                        [128,4] pooled — lands directly in [dff, b] layout)
      GEMM1           h^T tiles [128 dff, 512 tok]; token-major inner loop
                      so the gelu (ACT) of token-chunk tb drains its PSUM
                      bank 24 matmuls before the next c-chunk needs it
      GEMM2           one [128 tok, 1024 d] PSUM tile (2 banks) per token
                      tile; a single DVE add accumulates it into the SBUF
                      fp32 accumulator (halves DVE instruction pressure)
  - x is staged as 4 per-token-block tiles so GEMM1 starts on block 0
    as soon as the first 1MB of x has landed.
"""
from contextlib import ExitStack

import ml_dtypes
import numpy as np

import concourse.bass as bass
import concourse.tile as tile
from concourse import bacc, mybir
from concourse.bass_utils import run_bass_kernel_spmd

F32 = mybir.dt.float32
BF16 = mybir.dt.bfloat16
AF = mybir.ActivationFunctionType

B, E, N, D = 4, 8, 512, 1024
DFF = 4 * D
NA = 256
TOK = B * N            # 2048 tokens per expert
KC = D // 128          # 8 d-chunks
NDFB = 8               # dff blocks
DFB = DFF // NDFB      # 512
NCC = DFB // 128       # 4 c-chunks per block
NTB = 4                # token chunks of 512 (== batch b)
NTS = TOK // 128       # 16 token chunks of 128
NC_CORES = 8

_cache = {}


def _build():
    nc = bacc.Bacc("TRN2", target_bir_lowering=False, debug=False,
                   num_devices=NC_CORES)

    xT_d = nc.declare_dram_parameter("xT", [NTB, 128, KC, N], BF16, isOutput=False)
    apT_d = nc.declare_dram_parameter("apT", [128, KC, B], BF16, isOutput=False)
    w1_d = nc.declare_dram_parameter("w1", [NDFB, 128, KC, DFB], BF16, isOutput=False)
    wa_d = nc.declare_dram_parameter("wa", [NDFB, 128, KC, DFB], BF16, isOutput=False)
    w2_d = nc.declare_dram_parameter("w2", [NDFB, 128, NCC, D], BF16, isOutput=False)
    b1t_d = nc.declare_dram_parameter("b1t", [128, DFF // 128], F32, isOutput=False)
    b2b_d = nc.declare_dram_parameter("b2b", [128, D], F32, isOutput=False)
    out_d = nc.declare_dram_parameter("out", [TOK, D], F32, isOutput=True)

    with tile.TileContext(nc) as tc, ExitStack() as ctx:
        sb = ctx.enter_context(tc.tile_pool(name="sb", bufs=1))
        ps = ctx.enter_context(
            tc.tile_pool(name="ps", bufs=1, space=bass.MemorySpace.PSUM))

        # ---- small persistent tiles -------------------------------------
        apT_t = sb.tile([128, KC, B], BF16, name="apT_t")
        b1t_t = sb.tile([128, DFF // 128], F32, name="b1t_t")
        b2b_t = sb.tile([128, D], F32, name="b2b_t")
        baud_t = sb.tile([128, DFF // 128, B], F32, name="baud_t")
        nc.sync.dma_start(out=apT_t[:], in_=apT_d.ap())
        nc.gpsimd.dma_start(out=b1t_t[:], in_=b1t_d.ap())
        nc.gpsimd.dma_start(out=b2b_t[:], in_=b2b_d.ap())

        # ---- DMA helpers (one contiguous 8KB/partition load per call) ---
        def dma_w(which, d_param, blk, shape):
            t = sb.tile(shape, BF16, name=f"{which}_{blk}", tag=which, bufs=2)
            nc.sync.dma_start(out=t[:], in_=d_param.ap()[blk])
            return t

        # ---- start-up: hand-ordered DMA queue ---------------------------
        # The start is DMA-throughput-bound, so block 0's tiles are split
        # fine-grained and ordered exactly along the consumption order of
        # the PE stream: wa0 (audio) -> w1 c0-chunk + xT tb0 (GEMM1 c0/tb0)
        # -> remaining token blocks -> rest of w1 -> w2.
        wa_t = dma_w("wa", wa_d, 0, [128, KC, DFB])
        w1c0_t = sb.tile([128, KC, 128], BF16, name="w1c0_t")
        nc.sync.dma_start(out=w1c0_t[:], in_=w1_d.ap()[0][:, :, 0:128])
        xT_t = []
        xT0h = []
        for h in range(2):
            t = sb.tile([128, KC // 2, N], BF16, name=f"xT_0_{h}")
            nc.sync.dma_start(out=t[:], in_=xT_d.ap()[0][:, h * 4:(h + 1) * 4, :])
            xT0h.append(t)
        xT_t.append(None)  # tb0 handled via xT0h
        for tb in range(1, NTB):
            t = sb.tile([128, KC, N], BF16, name=f"xT_{tb}")
            nc.sync.dma_start(out=t[:], in_=xT_d.ap()[tb])
            xT_t.append(t)
        w1r_t = sb.tile([128, KC, DFB - 128], BF16, name="w1r_t")
        nc.sync.dma_start(out=w1r_t[:], in_=w1_d.ap()[0][:, :, 128:DFB])
        w2_t = dma_w("w2", w2_d, 0, [128, NCC, D])

        def xT_ap(tb, kc):
            if tb == 0:
                return xT0h[kc // 4][:, kc % 4, :]
            return xT_t[tb][:, kc, :]

        def w1_ap(blk, w1t, kc, c):
            if blk == 0:
                if c == 0:
                    return w1c0_t[:, kc, :]
                return w1r_t[:, kc, (c - 1) * 128:c * 128]
            return w1t[:, kc, c * 128:(c + 1) * 128]

        # ---- PE warm-up -------------------------------------------------
        # Cover the DMA-bound first ~7us with throwaway matmuls so the HAM
        # clock gate is released by the time real work arrives; the dummy
        # activation pulls the one-time ~2.6us gelu table load off the
        # first real GEMM1 chunk's critical path.
        scr_t = sb.tile([128, 4], F32, name="scr_t")
        nc.scalar.activation(scr_t[:], apT_t[:, 0, :], AF.Gelu_apprx_tanh,
                             scale=1.0)
        psW = ps.tile([B, B], F32, name="psW", tag="ps2", bufs=2)
        for _ in range(168):
            nc.tensor.matmul(psW[:], apT_t[:, 0, :], apT_t[:, 0, :],
                             start=True, stop=True)

        # ---- main loop --------------------------------------------------
        oacc = [sb.tile([128, D], F32, name=f"oacc_{t}", tag=f"oacc{t}",
                        bufs=1) for t in range(NTS)]
        w1_t = None  # block 0 reads via w1c0_t / w1r_t
        for blk in range(NDFB):
            first_blk = blk == 0
            last_blk = blk == NDFB - 1
            if not first_blk:
                wa_t = dma_w("wa", wa_d, blk, [128, KC, DFB])
                w1_t = dma_w("w1", w1_d, blk, [128, KC, DFB])
                w2_t = dma_w("w2", w2_d, blk, [128, NCC, D])

            # audio bias: baud[:, cg, b] = (Wa_blk^T @ pooled^T)[dff, b] + b1
            for c in range(NCC):
                cg = blk * NCC + c
                psB = ps.tile([128, B], F32, name=f"psB{cg}", tag="ps2",
                              bufs=2)
                for kc in range(KC):
                    nc.tensor.matmul(
                        psB[:], wa_t[:, kc, c * 128:(c + 1) * 128],
                        apT_t[:, kc, :],
                        start=(kc == 0), stop=(kc == KC - 1))
                nc.vector.tensor_scalar_add(
                    baud_t[:, cg, :], psB[:], b1t_t[:, cg:cg + 1])

            # GEMM1: h^T tiles [128 dff, 512 tok]; token-major inner order
            hT = []
            for c in range(NCC):
                cg = blk * NCC + c
                p1 = [ps.tile([128, N], F32, name=f"ps1_{blk}_{c}_{tb}",
                              tag=f"ps1{tb}", bufs=1) for tb in range(NTB)]
                row = []
                for tb in range(NTB):
                    for kc in range(KC):
                        nc.tensor.matmul(
                            p1[tb][:], w1_ap(blk, w1_t, kc, c),
                            xT_ap(tb, kc),
                            start=(kc == 0), stop=(kc == KC - 1))
                    h = sb.tile([128, N], BF16, name=f"hT_{blk}_{c}_{tb}",
                                tag=f"hT{c}b{tb}", bufs=2)
                    nc.scalar.activation(
                        h[:], p1[tb][:], AF.Gelu_apprx_tanh,
                        bias=baud_t[:, cg, tb:tb + 1], scale=1.0)
                    row.append(h)
                hT.append(row)

            # GEMM2: one [128 tok, 1024 d] PSUM tile (2 banks) per tsg
            for tsg in range(NTS):
                tb, r = tsg // 4, tsg % 4
                tail = last_blk and tsg == NTS - 1
                if not tail:
                    p2 = ps.tile([128, D], F32, name=f"ps2_{blk}_{tsg}",
                                 tag="ps2", bufs=2)
                    halves = [p2[:, 0:512], p2[:, 512:1024]]
                else:
                    # final tile: two 1-bank tiles in GEMM1's (now idle)
                    # banks so the first half drains+stores while the
                    # second half is still on the PE
                    pt = [ps.tile([128, 512], F32, name=f"ps2t_{dh}",
                                  tag=f"ps1{dh}", bufs=1) for dh in range(2)]
                    halves = [pt[0][:], pt[1][:]]
                for dh in range(2):
                    for c in range(NCC):
                        nc.tensor.matmul(
                            halves[dh], hT[c][tb][:, r * 128:(r + 1) * 128],
                            w2_t[:, c, dh * 512:(dh + 1) * 512],
                            start=(c == 0), stop=(c == NCC - 1))
                    if tail:
                        dst = oacc[tsg][:, dh * 512:(dh + 1) * 512]
                        nc.vector.tensor_add(dst, dst, halves[dh])
                        row0 = tsg * 128
                        nc.scalar.dma_start(
                            out=out_d.ap()[row0:row0 + 128,
                                           dh * 512:(dh + 1) * 512],
                            in_=dst)
                if not tail:
                    if first_blk:
                        nc.vector.tensor_add(oacc[tsg][:], p2[:], b2b_t[:])
                    else:
                        nc.vector.tensor_add(oacc[tsg][:], oacc[tsg][:], p2[:])
                    if last_blk:
                        row0 = tsg * 128
                        nc.scalar.dma_start(
                            out=out_d.ap()[row0:row0 + 128, :],
                            in_=oacc[tsg][:])

    nc.compile()
    return nc


def _get_nc():
    if "nc" not in _cache:
        _cache["nc"] = _build()
    return _cache["nc"]


def _prep_in_maps(x, audio_feat, W1, b1, Wa, W2, b2):
    bf = ml_dtypes.bfloat16
    pooled = audio_feat.mean(axis=1)                          # [B, D]
    apT = np.ascontiguousarray(
        pooled.T.reshape(KC, 128, B).transpose(1, 0, 2)).astype(bf)
    in_maps = []
    for e in range(E):
        xT = np.ascontiguousarray(
            x[:, e].reshape(TOK, D).astype(bf)
            .reshape(NTB, N, KC, 128).transpose(0, 3, 2, 1))
        w1 = np.ascontiguousarray(
            W1[e].astype(bf).reshape(KC, 128, NDFB, DFB).transpose(2, 1, 0, 3))
        wa = np.ascontiguousarray(
            Wa[e].astype(bf).reshape(KC, 128, NDFB, DFB).transpose(2, 1, 0, 3))
        w2 = np.ascontiguousarray(
            W2[e].astype(bf).reshape(NDFB, NCC, 128, D).transpose(0, 2, 1, 3))
        in_maps.append({
            "xT": xT,
            "apT": apT,
            "w1": w1,
            "wa": wa,
            "w2": w2,
            "b1t": np.ascontiguousarray(b1[e].reshape(DFF // 128, 128).T),
            "b2b": np.ascontiguousarray(np.broadcast_to(b2[e], (128, D))),
        })
    return in_maps


def kernel(x, audio_feat, W1, b1, Wa, W2, b2):
    x = np.asarray(x, dtype=np.float32)
    audio_feat = np.asarray(audio_feat, dtype=np.float32)
    W1 = np.asarray(W1, dtype=np.float32)
    b1 = np.asarray(b1, dtype=np.float32)
    Wa = np.asarray(Wa, dtype=np.float32)
    W2 = np.asarray(W2, dtype=np.float32)
    b2 = np.asarray(b2, dtype=np.float32)

    nc = _get_nc()
    in_maps = _prep_in_maps(x, audio_feat, W1, b1, Wa, W2, b2)
    _cache["in_maps"] = in_maps

    # A prior tenant can leave the accelerator in an unrecoverable state
    # that clears after one failed attempt; retry to absorb that.
    last_err = None
    for attempt in range(3):
        try:
            res = run_bass_kernel_spmd(nc, in_maps, list(range(NC_CORES)))
            break
        except Exception as err:  # noqa: BLE001
            last_err = err
            import time
            time.sleep(2.0)
    else:
        raise last_err

    out = np.empty((B, E, N, D), dtype=np.float32)
    for e in range(E):
        out[:, e] = res.results[e]["out"].reshape(B, N, D)
    return out
